# revision 24
# baseline (speedup 1.0000x reference)
"""Kernel for nn_Attention_F_12214886990460.

Full-input contract: kernel(**inputs) takes the complete (unsharded) numpy
inputs and returns the full (4, 256, 128, 128) float32 output.

Algebraic restructurings (exact up to f32 rounding; validated against the
jax reference):

  * Imag Gram is identically zero: for real x, Im(sum_n xf_c[n]*xf_d[n]) = 0
    by conjugate symmetry.  The reference's imag softmax therefore acts on
    float noise whose magnitude after the 1/(|q_c||q_d|) scaling is ~1e-7,
    so its output is the uniform matrix 1/32 to ~1e-8 — closed form used
    directly, skipping one GEMM + softmax per batch.
  * Real Gram from the rfft2 half-spectrum with column weights
    (w=1 for kw in {0, W/2}, w=2 otherwise), using hermitian symmetry of xf:
    G = (R*w) R^T - (I*w) I^T over 65 of 128 columns.
  * Row norms via Parseval: |q_c|^2 = HW * sum x_c^2.
  * The gate is a pointwise function of Re(xf) (conjugate-symmetric), so
    gate*xf is hermitian: the gating branch runs on the half spectrum and
    returns through irfft2 — half the FFT work, exactly-real ifft2.
  * Channel-axis IDFT32 folded into the attention weights (M = D32 @ attn);
    D32 @ (uniform imag part) has the closed form delta_{c,0}/32.  The
    attention apply runs as 4 real batched SGEMMs on separate R/I planes;
    full-spectrum R/I are mirror-reconstructed with pure strided copies.
  * The final 1x1 projection (the largest GEMM) runs as a bf16 AMX matmul
    via torch (fp32 accumulation); everything upstream stays f32.
"""

import numpy as np

try:
    # Raise glibc's mmap threshold so the large per-batch temporaries
    # (scipy FFT work buffers, numpy temps: 16-34 MiB each) are served from
    # the reusable heap free-list instead of fresh mmap'd pages that the
    # kernel must zero on every fault.  M_MMAP_THRESHOLD == -3.
    import ctypes
    ctypes.CDLL("libc.so.6", use_errno=True).mallopt(-3, 1 << 30)
except Exception:  # pragma: no cover
    pass

try:
    import scipy.fft as _sfft
except Exception:  # pragma: no cover
    _sfft = None

try:
    import torch
    torch.set_num_threads(1)
    # smoke-test the bf16 matmul path used for the final projection
    _t = (torch.ones(2, 2, dtype=torch.bfloat16)
          @ torch.ones(2, 2, dtype=torch.bfloat16)).float().numpy()
    _HAS_TORCH = bool(np.allclose(_t, 2.0))
except Exception:  # pragma: no cover
    _HAS_TORCH = False

NUM_HEADS = 8
BN_EPS = 1e-5
NORM_EPS = 1e-12

B, C, H, W = 4, 256, 128, 128
HD = NUM_HEADS
CPH = C // HD           # 32 channels per head
N = H * W               # 16384
KH = W // 2 + 1         # 65 rfft columns

_k32 = np.arange(CPH)
_D32 = (np.exp(+2j * np.pi * np.outer(_k32, _k32) / CPH) / CPH).astype(
    np.complex64)        # scaled IDFT32
_D32r = np.ascontiguousarray(_D32.real)
_D32i = np.ascontiguousarray(_D32.imag)

# hermitian column weights for half-spectrum inner products
_CW = np.full(KH, 2.0, dtype=np.float32)
_CW[0] = 1.0
_CW[KH - 1] = 1.0


def _rfft2(a):
    if _sfft is not None:
        return _sfft.rfft2(a)
    return np.fft.rfft2(a).astype(np.complex64)


def _irfft2(a):
    if _sfft is not None:
        return _sfft.irfft2(a, s=(H, W), overwrite_x=True)
    return np.fft.irfft2(a, s=(H, W)).astype(np.float32)


def _ifft(a):
    if _sfft is not None:
        return _sfft.ifft(a, axis=-1, overwrite_x=True)
    return np.fft.ifft(a, axis=-1).astype(np.complex64)


def _softmax(m):
    e = np.exp(m - m.max(axis=-1, keepdims=True))
    e /= e.sum(axis=-1, keepdims=True)
    return e


def _mirror(dst, half):
    """dst[:, :, KH:] = conj-mirror of half (one sign-adjusted strided copy).

    dst[c, kh, kw'] = half[c, (-kh) % H, W - kw'] for kw' in [KH, W).
    Row 0 maps to row 0; rows 1.. map to reversed rows — both plain slices.
    """
    dst[..., 0, KH:] = half[..., 0, KH - 2:0:-1]
    dst[..., 1:, KH:] = half[..., :0:-1, KH - 2:0:-1]


def _mirror_neg(dst, half):
    """Same as _mirror but writes the negated mirror (fused conjugate)."""
    np.negative(half[..., 0, KH - 2:0:-1], out=dst[..., 0, KH:])
    np.negative(half[..., :0:-1, KH - 2:0:-1], out=dst[..., 1:, KH:])


def kernel(x, temperature, w1, b1, bn_gamma, bn_beta, bn_mean, bn_var,
           w2, b2, proj_w):
    x = np.asarray(x, dtype=np.float32)
    temp = np.asarray(temperature, dtype=np.float32).reshape(HD, 1, 1)
    w1 = np.asarray(w1, dtype=np.float32)
    b1 = np.asarray(b1, dtype=np.float32)
    bn_gamma = np.asarray(bn_gamma, dtype=np.float32)
    bn_beta = np.asarray(bn_beta, dtype=np.float32)
    bn_mean = np.asarray(bn_mean, dtype=np.float32)
    bn_var = np.asarray(bn_var, dtype=np.float32)
    w2 = np.asarray(w2, dtype=np.float32)
    b2 = np.asarray(b2, dtype=np.float32)
    proj_w = np.asarray(proj_w, dtype=np.float32)

    if _HAS_TORCH:
        pT = torch.from_numpy(np.ascontiguousarray(proj_w)).bfloat16()
    pA = np.ascontiguousarray(proj_w[:, :C])
    pB = np.ascontiguousarray(proj_w[:, C:])

    # fold BN (inference) + conv bias b1 into w1 itself: relu(bn(w1@x+b1))
    # == relu(w1s@x + bn_b) with w1s = bn_a*w1
    bn_a = bn_gamma / np.sqrt(bn_var + BN_EPS)
    bn_b = (bn_beta - bn_mean * bn_a + bn_a * b1).astype(np.float32)
    w1s = (w1 * bn_a[:, None]).astype(np.float32)

    out = np.empty((B, C, H, W), dtype=np.float32)

    # preallocate large per-batch buffers once (avoids 100+ MiB of fresh
    # mmap + page-zeroing per batch)
    Rh = np.empty((C, H, KH), dtype=np.float32)
    Ih = np.empty((C, H, KH), dtype=np.float32)
    RIf = np.empty((HD, 2 * CPH, H, W), dtype=np.float32)
    MM = np.empty((HD, 2 * CPH, 2 * CPH), dtype=np.float32)
    o2 = np.empty((HD, 2 * CPH, N), dtype=np.float32)
    out2 = np.empty((C, N), dtype=np.complex64)
    absbuf = np.empty((C, N), dtype=np.float32)
    if _HAS_TORCH:
        cat = torch.empty((2 * C, N), dtype=torch.bfloat16)

    with np.errstate(over="ignore"):
        for b in range(B):
            xb = x[b]                                     # (256,128,128) f32

            # ---- forward half-spectrum FFT ----
            xfh = _rfft2(xb)                              # (256,128,65) c64
            np.copyto(Rh, xfh.real)
            np.copyto(Ih, xfh.imag)

            # ---- Gram from half spectrum (real part only) ----
            # sum over full spectrum = 2*sum(half) - endpoint columns
            # (kw=0 and kw=W/2 appear once, interior columns twice)
            Rm = Rh.reshape(HD, CPH, -1)
            Im_ = Ih.reshape(HD, CPH, -1)
            G1 = np.matmul(Rm, Rm.transpose(0, 2, 1))     # sum R*R (half)
            G2 = np.matmul(Im_, Im_.transpose(0, 2, 1))   # sum I*I (half)
            G = G1 - G2
            nrm2 = np.einsum('hcc->hc', G1) + np.einsum('hcc->hc', G2)
            G *= 2.0
            nrm2 *= 2.0
            for kw in (0, KH - 1):
                Re_ = np.ascontiguousarray(Rh[:, :, kw]).reshape(HD, CPH, H)
                Ie_ = np.ascontiguousarray(Ih[:, :, kw]).reshape(HD, CPH, H)
                C1 = np.matmul(Re_, Re_.transpose(0, 2, 1))
                C2 = np.matmul(Ie_, Ie_.transpose(0, 2, 1))
                G -= C1 - C2
                nrm2 -= np.einsum('hcc->hc', C1) + np.einsum('hcc->hc', C2)
            nrm = np.sqrt(nrm2)
            inv = (1.0 / np.maximum(nrm, NORM_EPS)).astype(np.float32)
            scale = inv[:, :, None] * inv[:, None, :]
            ar = _softmax(G * scale * temp)
            Mr = np.matmul(_D32r, ar)                     # (8,32,32)
            Mi = np.matmul(_D32i, ar)
            Mi[:, 0, :] += np.float32(1.0 / CPH)          # D32 @ (1/32) term

            # ---- mirror half -> full spectrum, stacked [R; I] per head ----
            Rf = RIf[:, :CPH]                             # views into RIf
            If = RIf[:, CPH:]
            Rh4 = Rh.reshape(HD, CPH, H, KH)
            Ih4 = Ih.reshape(HD, CPH, H, KH)
            Rf[..., :KH] = Rh4
            If[..., :KH] = Ih4
            _mirror(Rf, Rh4)
            _mirror_neg(If, Ih4)                          # fused conjugate

            # ---- attention apply as ONE batched sgemm ----
            # [o2r; o2i] = [[Mr, -Mi], [Mi, Mr]] @ [Rq; Iq]
            MM[:, :CPH, :CPH] = Mr
            MM[:, :CPH, CPH:] = -Mi
            MM[:, CPH:, :CPH] = Mi
            MM[:, CPH:, CPH:] = Mr
            np.matmul(MM, RIf.reshape(HD, 2 * CPH, N), out=o2)
            o2r = o2[:, :CPH].reshape(C, N)
            o2i = o2[:, CPH:].reshape(C, N)
            for c0 in range(0, C, 16):   # chunked: re+im writes share L2
                c1 = c0 + 16
                out2.real[c0:c1] = o2r[c0:c1]
                out2.imag[c0:c1] = o2i[c0:c1]

            oi = _ifft(out2)                              # (256,16384) c64

            # ---- gating branch on the half spectrum ----
            xrh = Rh.reshape(C, -1)                       # (256,8320)
            y = w1s @ xrh
            y += bn_b[:, None]
            np.maximum(y, 0.0, out=y)
            y2 = w2 @ y
            y2 += b2[:, None]
            np.negative(y2, out=y2)
            np.exp(y2, out=y2)
            y2 += 1.0
            np.reciprocal(y2, out=y2)                     # sigmoid
            xfh *= y2.reshape(C, H, KH)                   # gate, in place
            out_l = _irfft2(xfh)                          # exact real ifft2

            # ---- final 1x1 projection (bf16 AMX via torch if present) ----
            if _HAS_TORCH:
                np.abs(oi, out=absbuf)
                cat[:C] = torch.from_numpy(absbuf)
                np.abs(out_l.reshape(C, N), out=absbuf)
                cat[C:] = torch.from_numpy(absbuf)
                res = pT @ cat                            # bf16 AMX matmul
                ov = torch.from_numpy(out[b].reshape(C, N))
                ov.copy_(res)                             # cast into output
            else:  # pragma: no cover
                r = pA @ np.abs(oi)
                r += pB @ np.abs(out_l).reshape(C, N)
                out[b] = r.reshape(C, H, W)

    return out


# revision 25
# speedup vs baseline: 1.2203x; 1.2203x over previous
"""Kernel for nn_Attention_F_12214886990460.

Full-input contract: kernel(**inputs) takes the complete (unsharded) numpy
inputs and returns the full (4, 256, 128, 128) float32 output.

Algebraic restructurings (exact up to f32 rounding; validated against the
jax reference):

  * Imag Gram is identically zero: for real x, Im(sum_n xf_c[n]*xf_d[n]) = 0
    by conjugate symmetry.  The reference's imag softmax therefore acts on
    float noise whose magnitude after the 1/(|q_c||q_d|) scaling is ~1e-7,
    so its output is the uniform matrix 1/32 to ~1e-8 — closed form used
    directly, skipping one GEMM + softmax per batch.
  * Real Gram from the rfft2 half-spectrum with column weights
    (w=1 for kw in {0, W/2}, w=2 otherwise), using hermitian symmetry of xf:
    G = (R*w) R^T - (I*w) I^T over 65 of 128 columns.
  * Row norms via Parseval: |q_c|^2 = HW * sum x_c^2.
  * The gate is a pointwise function of Re(xf) (conjugate-symmetric), so
    gate*xf is hermitian: the gating branch runs on the half spectrum and
    returns through irfft2 — half the FFT work, exactly-real ifft2.
  * Channel-axis IDFT32 folded into the attention weights (M = D32 @ attn);
    D32 @ (uniform imag part) has the closed form delta_{c,0}/32.  The
    attention apply runs as 4 real batched SGEMMs on separate R/I planes;
    full-spectrum R/I are mirror-reconstructed with pure strided copies.
  * The final 1x1 projection (the largest GEMM) runs as a bf16 AMX matmul
    via torch (fp32 accumulation); everything upstream stays f32.
"""

import numpy as np

try:
    # Raise glibc's mmap threshold so the large per-batch temporaries
    # (scipy FFT work buffers, numpy temps: 16-34 MiB each) are served from
    # the reusable heap free-list instead of fresh mmap'd pages that the
    # kernel must zero on every fault.  M_MMAP_THRESHOLD == -3.
    import ctypes
    ctypes.CDLL("libc.so.6", use_errno=True).mallopt(-3, 1 << 30)
except Exception:  # pragma: no cover
    pass

try:
    import scipy.fft as _sfft
except Exception:  # pragma: no cover
    _sfft = None

try:
    import torch
    torch.set_num_threads(1)
    # smoke-test the bf16 matmul path used for the final projection
    _t = (torch.ones(2, 2, dtype=torch.bfloat16)
          @ torch.ones(2, 2, dtype=torch.bfloat16)).float().numpy()
    _HAS_TORCH = bool(np.allclose(_t, 2.0))
except Exception:  # pragma: no cover
    _HAS_TORCH = False

NUM_HEADS = 8
BN_EPS = 1e-5
NORM_EPS = 1e-12

B, C, H, W = 4, 256, 128, 128
HD = NUM_HEADS
CPH = C // HD           # 32 channels per head
N = H * W               # 16384
KH = W // 2 + 1         # 65 rfft columns

_k32 = np.arange(CPH)
_D32 = (np.exp(+2j * np.pi * np.outer(_k32, _k32) / CPH) / CPH).astype(
    np.complex64)        # scaled IDFT32
_D32r = np.ascontiguousarray(_D32.real)
_D32i = np.ascontiguousarray(_D32.imag)

# hermitian column weights for half-spectrum inner products
_CW = np.full(KH, 2.0, dtype=np.float32)
_CW[0] = 1.0
_CW[KH - 1] = 1.0


def _rfft2(a):
    if _sfft is not None:
        return _sfft.rfft2(a)
    return np.fft.rfft2(a).astype(np.complex64)


def _irfft2(a):
    if _sfft is not None:
        return _sfft.irfft2(a, s=(H, W), overwrite_x=True)
    return np.fft.irfft2(a, s=(H, W)).astype(np.float32)


def _ifft(a):
    if _sfft is not None:
        return _sfft.ifft(a, axis=-1, overwrite_x=True)
    return np.fft.ifft(a, axis=-1).astype(np.complex64)


def _softmax(m):
    e = np.exp(m - m.max(axis=-1, keepdims=True))
    e /= e.sum(axis=-1, keepdims=True)
    return e


def _mirror(dst, half):
    """dst[:, :, KH:] = conj-mirror of half (one sign-adjusted strided copy).

    dst[c, kh, kw'] = half[c, (-kh) % H, W - kw'] for kw' in [KH, W).
    Row 0 maps to row 0; rows 1.. map to reversed rows — both plain slices.
    """
    dst[..., 0, KH:] = half[..., 0, KH - 2:0:-1]
    dst[..., 1:, KH:] = half[..., :0:-1, KH - 2:0:-1]


def _mirror_neg(dst, half):
    """Same as _mirror but writes the negated mirror (fused conjugate)."""
    np.negative(half[..., 0, KH - 2:0:-1], out=dst[..., 0, KH:])
    np.negative(half[..., :0:-1, KH - 2:0:-1], out=dst[..., 1:, KH:])


def kernel(x, temperature, w1, b1, bn_gamma, bn_beta, bn_mean, bn_var,
           w2, b2, proj_w):
    x = np.asarray(x, dtype=np.float32)
    temp = np.asarray(temperature, dtype=np.float32).reshape(HD, 1, 1)
    w1 = np.asarray(w1, dtype=np.float32)
    b1 = np.asarray(b1, dtype=np.float32)
    bn_gamma = np.asarray(bn_gamma, dtype=np.float32)
    bn_beta = np.asarray(bn_beta, dtype=np.float32)
    bn_mean = np.asarray(bn_mean, dtype=np.float32)
    bn_var = np.asarray(bn_var, dtype=np.float32)
    w2 = np.asarray(w2, dtype=np.float32)
    b2 = np.asarray(b2, dtype=np.float32)
    proj_w = np.asarray(proj_w, dtype=np.float32)

    if _HAS_TORCH:
        pT = torch.from_numpy(np.ascontiguousarray(proj_w)).bfloat16()
    pA = np.ascontiguousarray(proj_w[:, :C])
    pB = np.ascontiguousarray(proj_w[:, C:])

    # fold BN (inference) + conv bias b1 into w1 itself: relu(bn(w1@x+b1))
    # == relu(w1s@x + bn_b) with w1s = bn_a*w1
    bn_a = bn_gamma / np.sqrt(bn_var + BN_EPS)
    bn_b = (bn_beta - bn_mean * bn_a + bn_a * b1).astype(np.float32)
    w1s = (w1 * bn_a[:, None]).astype(np.float32)

    out = np.empty((B, C, H, W), dtype=np.float32)

    # preallocate large per-batch buffers once (avoids 100+ MiB of fresh
    # mmap + page-zeroing per batch)
    Rh = np.empty((C, H, KH), dtype=np.float32)
    Ih = np.empty((C, H, KH), dtype=np.float32)
    RIf = np.empty((HD, 2 * CPH, H, W), dtype=np.float32)
    MM = np.empty((HD, 2 * CPH, 2 * CPH), dtype=np.float32)
    o2 = np.empty((HD, 2 * CPH, N), dtype=np.float32)
    out2 = np.empty((C, N), dtype=np.complex64)
    absbuf = np.empty((C, N), dtype=np.float32)
    if _HAS_TORCH:
        cat = torch.empty((2 * C, N), dtype=torch.bfloat16)

    with np.errstate(over="ignore"):
        for b in range(B):
            xb = x[b]                                     # (256,128,128) f32

            # ---- forward half-spectrum FFT ----
            xfh = _rfft2(xb)                              # (256,128,65) c64
            np.copyto(Rh, xfh.real)
            np.copyto(Ih, xfh.imag)

            # ---- Gram from half spectrum (real part only) ----
            # sum over full spectrum = 2*sum(half) - endpoint columns
            # (kw=0 and kw=W/2 appear once, interior columns twice)
            Rm = Rh.reshape(HD, CPH, -1)
            Im_ = Ih.reshape(HD, CPH, -1)
            G1 = np.matmul(Rm, Rm.transpose(0, 2, 1))     # sum R*R (half)
            G2 = np.matmul(Im_, Im_.transpose(0, 2, 1))   # sum I*I (half)
            G = G1 - G2
            nrm2 = np.einsum('hcc->hc', G1) + np.einsum('hcc->hc', G2)
            G *= 2.0
            nrm2 *= 2.0
            for kw in (0, KH - 1):
                Re_ = np.ascontiguousarray(Rh[:, :, kw]).reshape(HD, CPH, H)
                Ie_ = np.ascontiguousarray(Ih[:, :, kw]).reshape(HD, CPH, H)
                C1 = np.matmul(Re_, Re_.transpose(0, 2, 1))
                C2 = np.matmul(Ie_, Ie_.transpose(0, 2, 1))
                G -= C1 - C2
                nrm2 -= np.einsum('hcc->hc', C1) + np.einsum('hcc->hc', C2)
            nrm = np.sqrt(nrm2)
            inv = (1.0 / np.maximum(nrm, NORM_EPS)).astype(np.float32)
            scale = inv[:, :, None] * inv[:, None, :]
            ar = _softmax(G * scale * temp)
            Mr = np.matmul(_D32r, ar)                     # (8,32,32)
            Mi = np.matmul(_D32i, ar)
            Mi[:, 0, :] += np.float32(1.0 / CPH)          # D32 @ (1/32) term

            # ---- mirror half -> full spectrum, stacked [R; I] per head ----
            Rf = RIf[:, :CPH]                             # views into RIf
            If = RIf[:, CPH:]
            Rh4 = Rh.reshape(HD, CPH, H, KH)
            Ih4 = Ih.reshape(HD, CPH, H, KH)
            Rf[..., :KH] = Rh4
            If[..., :KH] = Ih4
            _mirror(Rf, Rh4)
            _mirror_neg(If, Ih4)                          # fused conjugate

            # ---- attention apply as ONE batched sgemm ----
            # [o2r; o2i] = [[Mr, -Mi], [Mi, Mr]] @ [Rq; Iq]
            MM[:, :CPH, :CPH] = Mr
            MM[:, :CPH, CPH:] = -Mi
            MM[:, CPH:, :CPH] = Mi
            MM[:, CPH:, CPH:] = Mr
            np.matmul(MM, RIf.reshape(HD, 2 * CPH, N), out=o2)
            o2r = o2[:, :CPH].reshape(C, N)
            o2i = o2[:, CPH:].reshape(C, N)
            for c0 in range(0, C, 4):    # chunked: re+im writes share L2
                c1 = c0 + 4
                out2.real[c0:c1] = o2r[c0:c1]
                out2.imag[c0:c1] = o2i[c0:c1]

            oi = _ifft(out2)                              # (256,16384) c64

            # ---- gating branch on the half spectrum ----
            xrh = Rh.reshape(C, -1)                       # (256,8320)
            y = w1s @ xrh
            y += bn_b[:, None]
            np.maximum(y, 0.0, out=y)
            y2 = w2 @ y
            y2 += b2[:, None]
            np.negative(y2, out=y2)
            np.exp(y2, out=y2)
            y2 += 1.0
            np.reciprocal(y2, out=y2)                     # sigmoid
            xfh *= y2.reshape(C, H, KH)                   # gate, in place
            out_l = _irfft2(xfh)                          # exact real ifft2

            # ---- final 1x1 projection (bf16 AMX via torch if present) ----
            if _HAS_TORCH:
                np.abs(oi, out=absbuf)
                cat[:C] = torch.from_numpy(absbuf)
                np.abs(out_l.reshape(C, N), out=absbuf)
                cat[C:] = torch.from_numpy(absbuf)
                res = pT @ cat                            # bf16 AMX matmul
                ov = torch.from_numpy(out[b].reshape(C, N))
                ov.copy_(res)                             # cast into output
            else:  # pragma: no cover
                r = pA @ np.abs(oi)
                r += pB @ np.abs(out_l).reshape(C, N)
                out[b] = r.reshape(C, H, W)

    return out
